# revision 1
# baseline (speedup 1.0000x reference)
import os
# Keep fp32 math exact on device: the CGAT LeakyReLU slope is 512, which
# amplifies any matmul downcast error straight through the softmax.
os.environ.setdefault("NEURON_CC_FLAGS", "--auto-cast=none")

import functools
import numpy as np
import jax
import jax.numpy as jnp

# dims (hardcoded from the problem spec)
B, V, T, F0, F1 = 8, 512, 12, 4, 64
G, K = 2, 2
H, DK, DV, DINNER = 4, 16, 16, 128
ALPHA = 0.2
ALPHA_CGAT = float(V)
NEG = -9e15
NCORES = 8
VSH = V // NCORES  # 64 v-rows per core for the encoder stage


def _leaky(x, a):
    return jnp.where(x >= 0, x, a * x)


def _device_fn(x, adjsub, idx, Ww, Wb, wt, aw, cWg, kvec, vstart,
               wq, wk, wv, fc, w1, w2):
    """Runs on ONE core. Computes one CGAT branch (g,k,offset), weighted by its
    cluster assignment; psum over the 8 cores yields the full weighted mean
    (the hint's all-reduce); then the encoder runs on this core's v-shard."""
    # ---- cluster softmax weight for this (g, k) ----
    xv = x.reshape(B, V, T * F0)
    logits = jnp.einsum('bvc,kc->bvk', xv, cWg)            # (B,V,K) for own g
    cl_g = jax.nn.softmax(logits, axis=-1)
    cl = jnp.einsum('bvk,k->bv', cl_g, kvec)               # (B,V) own k column

    # ---- CGAT branch (g, k, offset) ----
    h = _leaky(jnp.einsum('bvtf,of->bvto', x, Ww) + Wb, ALPHA_CGAT)  # (B,V,T,F1)
    ht = jnp.einsum('bvtf,t->vf', h, wt) / B                          # (V,F1)
    ha = jnp.take(h, idx, axis=1)                                     # (B,Va,T,F1)
    ht_a = jnp.take(ht, idx, axis=0)                                  # (Va,F1)
    e = _leaky((ht @ aw[F1:])[:, None] + (ht_a @ aw[:F1])[None, :], ALPHA_CGAT)
    scores = jnp.where(adjsub > 0, e, NEG)
    attn = jax.nn.softmax(scores, axis=-1)                            # (V,Va)
    br = _leaky(jnp.einsum('vu,butf->bvtf', attn, ha), ALPHA_CGAT)    # (B,V,T,F1)

    # weighted contribution; sum over all 8 cores = sum over (g,k,offset)
    y = br * (cl / G)[:, :, None, None]
    gc_act = jax.lax.psum(y, 'c')                                     # (B,V,T,F1)

    # ---- EncoderLayer on this core's v-shard ----
    qk = gc_act.mean(axis=1)                                          # (B,T,F1)
    q = (qk @ wq.T).reshape(B, T, H, DK)
    k = (qk @ wk.T).reshape(B, T, H, DK)
    scores2 = jnp.einsum('bqhd,bkhd->bhqk', q, k) / np.float32(np.sqrt(DK))
    attn2 = jax.nn.softmax(scores2, axis=-1)                          # (B,H,T,T)

    gcs = jax.lax.dynamic_slice_in_dim(gc_act, vstart, VSH, axis=1)   # (B,VSH,T,F1)
    vv = jnp.einsum('bvtf,of->bvto', gcs, wv).reshape(B, VSH, T, H, DV)
    out = jnp.einsum('bhqt,bnthd->bnqdh', attn2, vv).reshape(B, VSH, T, DV * H)
    out = _leaky(out @ fc.T, ALPHA)
    out = _leaky(_leaky(out @ w1.T, ALPHA) @ w2.T, ALPHA)
    return out                                                        # (B,VSH,T,F1)


_pmapped = jax.pmap(_device_fn, axis_name='c',
                    in_axes=(None, 0, 0, 0, 0, 0, 0, 0, 0, 0,
                             None, None, None, None, None, None))


@functools.lru_cache(maxsize=1)
def _branch_indices():
    # core c -> (g, k, offset); offsets interleave so (g,k,0)+(g,k,1) pairs sum
    return [(c // (K * 2), (c // 2) % K, c % 2) for c in range(NCORES)]


def kernel(x, graphs, cW, Ww0, Wb0, wt0, aw0, Ww1, Wb1, wt1, aw1,
           wq, wk, wv, fc, w1, w2):
    x = np.asarray(x, np.float32)
    graphs = np.asarray(graphs, np.float32)

    Wws = (np.asarray(Ww0, np.float32), np.asarray(Ww1, np.float32))
    Wbs = (np.asarray(Wb0, np.float32), np.asarray(Wb1, np.float32))
    wts = (np.asarray(wt0, np.float32), np.asarray(wt1, np.float32))
    aws = (np.asarray(aw0, np.float32), np.asarray(aw1, np.float32))

    # host-side shard prep (pure data staging, no model math)
    adjsub = np.empty((NCORES, V, V // 2), np.float32)
    idx = np.empty((NCORES, V // 2), np.int32)
    Ww_c = np.empty((NCORES, F1, F0), np.float32)
    Wb_c = np.empty((NCORES, F1), np.float32)
    wt_c = np.empty((NCORES, T), np.float32)
    aw_c = np.empty((NCORES, 2 * F1), np.float32)
    cWg_c = np.empty((NCORES, K, T * F0), np.float32)
    kvec_c = np.zeros((NCORES, K), np.float32)
    vstart_c = np.arange(NCORES, dtype=np.int32) * VSH

    cW = np.asarray(cW, np.float32)
    for c, (g, k, off) in enumerate(_branch_indices()):
        adjsub[c] = graphs[g][:, off::2]
        idx[c] = np.arange(off, V, 2, dtype=np.int32)
        Ww_c[c] = Wws[off][g, k]
        Wb_c[c] = Wbs[off][g, k]
        wt_c[c] = wts[off][g, k]
        aw_c[c] = aws[off][g, k]
        cWg_c[c] = cW[g]
        kvec_c[c, k] = 1.0

    res = _pmapped(x, adjsub, idx, Ww_c, Wb_c, wt_c, aw_c, cWg_c, kvec_c,
                   vstart_c,
                   np.asarray(wq, np.float32), np.asarray(wk, np.float32),
                   np.asarray(wv, np.float32), np.asarray(fc, np.float32),
                   np.asarray(w1, np.float32), np.asarray(w2, np.float32))
    res = np.asarray(res)                                  # (8, B, VSH, T, F1)
    return np.concatenate([res[c] for c in range(NCORES)], axis=1)


# revision 3
# speedup vs baseline: 1.0467x; 1.0467x over previous
import os
# Keep fp32 math exact on device: the CGAT LeakyReLU slope is 512, which
# amplifies any matmul downcast error straight through the softmax.
os.environ.setdefault("NEURON_CC_FLAGS", "--auto-cast=none")

import functools
import numpy as np
import jax
import jax.numpy as jnp

# dims (hardcoded from the problem spec)
B, V, T, F0, F1 = 8, 512, 12, 4, 64
G, K = 2, 2
H, DK, DV, DINNER = 4, 16, 16, 128
ALPHA = 0.2
ALPHA_CGAT = float(V)
NEG = -9e15
NCORES = 8
VSH = V // NCORES  # 64 v-rows per core for the encoder stage


def _leaky(x, a):
    return jnp.where(x >= 0, x, a * x)


def _device_fn(x, adjsub, idx, Ww, Wb, wt, aw, cWg, kvec, vstart,
               wq, wk, wv, fc, w1, w2):
    """Runs on ONE core. Computes one CGAT branch (g,k,offset), weighted by its
    cluster assignment; psum over the 8 cores yields the full weighted mean
    (the hint's all-reduce); then the encoder runs on this core's v-shard."""
    # ---- cluster softmax weight for this (g, k) ----
    xv = x.reshape(B, V, T * F0)
    logits = jnp.einsum('bvc,kc->bvk', xv, cWg)            # (B,V,K) for own g
    cl_g = jax.nn.softmax(logits, axis=-1)
    cl = jnp.einsum('bvk,k->bv', cl_g, kvec)               # (B,V) own k column

    # ---- CGAT branch (g, k, offset) ----
    h = _leaky(jnp.einsum('bvtf,of->bvto', x, Ww) + Wb, ALPHA_CGAT)  # (B,V,T,F1)
    ht = jnp.einsum('bvtf,t->vf', h, wt) / B                          # (V,F1)
    ha = jnp.take(h, idx, axis=1)                                     # (B,Va,T,F1)
    ht_a = jnp.take(ht, idx, axis=0)                                  # (Va,F1)
    e = _leaky((ht @ aw[F1:])[:, None] + (ht_a @ aw[:F1])[None, :], ALPHA_CGAT)
    scores = jnp.where(adjsub > 0, e, NEG)
    attn = jax.nn.softmax(scores, axis=-1)                            # (V,Va)
    br = _leaky(jnp.einsum('vu,butf->bvtf', attn, ha), ALPHA_CGAT)    # (B,V,T,F1)

    # weighted contribution; sum over all 8 cores = sum over (g,k,offset)
    y = br * (cl / G)[:, :, None, None]
    gc_act = jax.lax.psum(y, 'c')                                     # (B,V,T,F1)

    # ---- EncoderLayer on this core's v-shard ----
    qk = gc_act.mean(axis=1)                                          # (B,T,F1)
    q = (qk @ wq.T).reshape(B, T, H, DK)
    k = (qk @ wk.T).reshape(B, T, H, DK)
    scores2 = jnp.einsum('bqhd,bkhd->bhqk', q, k) / np.float32(np.sqrt(DK))
    attn2 = jax.nn.softmax(scores2, axis=-1)                          # (B,H,T,T)

    gcs = jax.lax.dynamic_slice_in_dim(gc_act, vstart, VSH, axis=1)   # (B,VSH,T,F1)
    vv = jnp.einsum('bvtf,of->bvto', gcs, wv).reshape(B, VSH, T, H, DV)
    out = jnp.einsum('bhqt,bnthd->bnqdh', attn2, vv).reshape(B, VSH, T, DV * H)
    out = _leaky(out @ fc.T, ALPHA)
    out = _leaky(_leaky(out @ w1.T, ALPHA) @ w2.T, ALPHA)
    return out                                                        # (B,VSH,T,F1)


_pmapped = jax.pmap(_device_fn, axis_name='c',
                    in_axes=(None, 0, 0, 0, 0, 0, 0, 0, 0, 0,
                             None, None, None, None, None, None))


@functools.lru_cache(maxsize=1)
def _branch_indices():
    # core c -> (g, k, offset); offsets interleave so (g,k,0)+(g,k,1) pairs sum
    return [(c // (K * 2), (c // 2) % K, c % 2) for c in range(NCORES)]


def kernel(x, graphs, cW, Ww0, Wb0, wt0, aw0, Ww1, Wb1, wt1, aw1,
           wq, wk, wv, fc, w1, w2):
    x = np.asarray(x, np.float32)
    graphs = np.asarray(graphs, np.float32)

    Wws = (np.asarray(Ww0, np.float32), np.asarray(Ww1, np.float32))
    Wbs = (np.asarray(Wb0, np.float32), np.asarray(Wb1, np.float32))
    wts = (np.asarray(wt0, np.float32), np.asarray(wt1, np.float32))
    aws = (np.asarray(aw0, np.float32), np.asarray(aw1, np.float32))

    # host-side shard prep (pure data staging, no model math)
    # adjacency ships as uint8: only its >0 predicate is used on device
    adjsub = np.empty((NCORES, V, V // 2), np.uint8)
    idx = np.empty((NCORES, V // 2), np.int32)
    Ww_c = np.empty((NCORES, F1, F0), np.float32)
    Wb_c = np.empty((NCORES, F1), np.float32)
    wt_c = np.empty((NCORES, T), np.float32)
    aw_c = np.empty((NCORES, 2 * F1), np.float32)
    cWg_c = np.empty((NCORES, K, T * F0), np.float32)
    kvec_c = np.zeros((NCORES, K), np.float32)
    vstart_c = np.arange(NCORES, dtype=np.int32) * VSH

    cW = np.asarray(cW, np.float32)
    for c, (g, k, off) in enumerate(_branch_indices()):
        adjsub[c] = (graphs[g][:, off::2] > 0).astype(np.uint8)
        idx[c] = np.arange(off, V, 2, dtype=np.int32)
        Ww_c[c] = Wws[off][g, k]
        Wb_c[c] = Wbs[off][g, k]
        wt_c[c] = wts[off][g, k]
        aw_c[c] = aws[off][g, k]
        cWg_c[c] = cW[g]
        kvec_c[c, k] = 1.0

    res = _pmapped(x, adjsub, idx, Ww_c, Wb_c, wt_c, aw_c, cWg_c, kvec_c,
                   vstart_c,
                   np.asarray(wq, np.float32), np.asarray(wk, np.float32),
                   np.asarray(wv, np.float32), np.asarray(fc, np.float32),
                   np.asarray(w1, np.float32), np.asarray(w2, np.float32))
    res = np.asarray(res)                                  # (8, B, VSH, T, F1)
    return np.concatenate([res[c] for c in range(NCORES)], axis=1)


# revision 4
# speedup vs baseline: 1.0665x; 1.0189x over previous
import os
# Keep fp32 math exact on device: the CGAT LeakyReLU slope is 512, which
# amplifies any matmul downcast error straight through the softmax.
os.environ.setdefault("NEURON_CC_FLAGS", "--auto-cast=none")

import numpy as np
import jax
import jax.numpy as jnp

# dims (hardcoded from the problem spec)
B, V, T, F0, F1 = 8, 512, 12, 4, 64
G, K = 2, 2
H, DK, DV, DINNER = 4, 16, 16, 128
ALPHA = 0.2
ALPHA_CGAT = float(V)
NEG = -9e15
NCORES = 8
VSH = V // NCORES  # 64 v-rows per core for the encoder stage

# packed per-core f32 param layout: Ww | Wb | wt | aw | cWg | kvec
_SZS = [F1 * F0, F1, T, 2 * F1, K * T * F0, K]
_OFF = np.cumsum([0] + _SZS).tolist()
# packed shared encoder weights: wq | wk | wv | fc | w1 | w2
_ESZS = [DK * H * F1, DK * H * F1, DV * H * F1, F1 * H * DV,
         DINNER * F1, F1 * DINNER]
_EOFF = np.cumsum([0] + _ESZS).tolist()


def _leaky(x, a):
    return jnp.where(x >= 0, x, a * x)


def _device_fn(x, adjsub, ints, pp, ew):
    """One core: one CGAT branch (g,k,offset) weighted by its cluster column;
    psum over the 8 cores gives the full weighted mean (the hint's all-reduce);
    the encoder then runs on this core's v-shard."""
    idx, vstart = ints[:V // 2], ints[V // 2]
    Ww = pp[_OFF[0]:_OFF[1]].reshape(F1, F0)
    Wb = pp[_OFF[1]:_OFF[2]]
    wt = pp[_OFF[2]:_OFF[3]]
    aw = pp[_OFF[3]:_OFF[4]]
    cWg = pp[_OFF[4]:_OFF[5]].reshape(K, T * F0)
    kvec = pp[_OFF[5]:_OFF[6]]
    wq = ew[_EOFF[0]:_EOFF[1]].reshape(H * DK, F1)
    wk = ew[_EOFF[1]:_EOFF[2]].reshape(H * DK, F1)
    wv = ew[_EOFF[2]:_EOFF[3]].reshape(H * DV, F1)
    fc = ew[_EOFF[3]:_EOFF[4]].reshape(F1, H * DV)
    w1 = ew[_EOFF[4]:_EOFF[5]].reshape(DINNER, F1)
    w2 = ew[_EOFF[5]:_EOFF[6]].reshape(F1, DINNER)

    # ---- cluster softmax weight for this (g, k) ----
    xv = x.reshape(B, V, T * F0)
    cl_g = jax.nn.softmax(jnp.einsum('bvc,kc->bvk', xv, cWg), axis=-1)
    cl = jnp.einsum('bvk,k->bv', cl_g, kvec)               # (B,V) own k column

    # ---- CGAT branch (g, k, offset) ----
    h = _leaky(jnp.einsum('bvtf,of->bvto', x, Ww) + Wb, ALPHA_CGAT)  # (B,V,T,F1)
    ht = jnp.einsum('bvtf,t->vf', h, wt) / B                          # (V,F1)
    ha = jnp.take(h, idx, axis=1)                                     # (B,Va,T,F1)
    ht_a = jnp.take(ht, idx, axis=0)                                  # (Va,F1)
    e = _leaky((ht @ aw[F1:])[:, None] + (ht_a @ aw[:F1])[None, :], ALPHA_CGAT)
    scores = jnp.where(adjsub > 0, e, NEG)
    attn = jax.nn.softmax(scores, axis=-1)                            # (V,Va)
    br = _leaky(jnp.einsum('vu,butf->bvtf', attn, ha), ALPHA_CGAT)    # (B,V,T,F1)

    # weighted contribution; sum over all 8 cores = sum over (g,k,offset)
    y = br * (cl / G)[:, :, None, None]
    gc_act = jax.lax.psum(y, 'c')                                     # (B,V,T,F1)

    # ---- EncoderLayer on this core's v-shard ----
    qk = gc_act.mean(axis=1)                                          # (B,T,F1)
    q = (qk @ wq.T).reshape(B, T, H, DK)
    k = (qk @ wk.T).reshape(B, T, H, DK)
    scores2 = jnp.einsum('bqhd,bkhd->bhqk', q, k) / np.float32(np.sqrt(DK))
    attn2 = jax.nn.softmax(scores2, axis=-1)                          # (B,H,T,T)

    gcs = jax.lax.dynamic_slice_in_dim(gc_act, vstart, VSH, axis=1)   # (B,VSH,T,F1)
    vv = jnp.einsum('bvtf,of->bvto', gcs, wv).reshape(B, VSH, T, H, DV)
    out = jnp.einsum('bhqt,bnthd->bnqdh', attn2, vv).reshape(B, VSH, T, DV * H)
    out = _leaky(out @ fc.T, ALPHA)
    out = _leaky(_leaky(out @ w1.T, ALPHA) @ w2.T, ALPHA)
    return out                                                        # (B,VSH,T,F1)


_pmapped = jax.pmap(_device_fn, axis_name='c',
                    in_axes=(None, 0, 0, 0, None))

_BRANCHES = [(c // (K * 2), (c // 2) % K, c % 2) for c in range(NCORES)]


def kernel(x, graphs, cW, Ww0, Wb0, wt0, aw0, Ww1, Wb1, wt1, aw1,
           wq, wk, wv, fc, w1, w2):
    x = np.asarray(x, np.float32)
    graphs = np.asarray(graphs, np.float32)
    Wws = (np.asarray(Ww0, np.float32), np.asarray(Ww1, np.float32))
    Wbs = (np.asarray(Wb0, np.float32), np.asarray(Wb1, np.float32))
    wts = (np.asarray(wt0, np.float32), np.asarray(wt1, np.float32))
    aws = (np.asarray(aw0, np.float32), np.asarray(aw1, np.float32))
    cW = np.asarray(cW, np.float32)

    # host-side shard prep (pure data staging, no model math)
    # adjacency ships as uint8: only its >0 predicate is used on device
    adjsub = np.empty((NCORES, V, V // 2), np.uint8)
    ints = np.empty((NCORES, V // 2 + 1), np.int32)
    pp = np.empty((NCORES, _OFF[-1]), np.float32)
    for c, (g, k, off) in enumerate(_BRANCHES):
        adjsub[c] = (graphs[g][:, off::2] > 0).astype(np.uint8)
        ints[c, :V // 2] = np.arange(off, V, 2, dtype=np.int32)
        ints[c, V // 2] = c * VSH
        kvec = np.zeros(K, np.float32)
        kvec[k] = 1.0
        pp[c] = np.concatenate([
            Wws[off][g, k].ravel(), Wbs[off][g, k], wts[off][g, k],
            aws[off][g, k], cW[g].ravel(), kvec])
    ew = np.concatenate([np.asarray(a, np.float32).ravel()
                         for a in (wq, wk, wv, fc, w1, w2)])

    res = np.asarray(_pmapped(x, adjsub, ints, pp, ew))   # (8,B,VSH,T,F1)
    return np.concatenate([res[c] for c in range(NCORES)], axis=1)
